# revision 17
# baseline (speedup 1.0000x reference)
"""DistMult scoring kernel v6 for Trainium2 (8 NeuronCores, SPMD 4x2 grid).

score = sigmoid( (ent_emb[h] * diag(rel_emb[r])) @ ent_emb[t].T )

v6 reshards the batch over a 4x2 core grid (4 head-groups x 2 tail-groups):
each core computes a [512, 1024] score block, gathering 4 head tiles + 8
tail tiles = 12 SWDGE calls/core (vs 18 for the 1x8 row sharding). The
SWDGE fixed overhead (~1.1us/call serialized on GpSimd) is THE wall, and
sqrt-sharding minimizes calls: 16/gh + 16/gt is minimal at (4,2).

Schedule (from v4/v5 trace analysis):
  - rels via one-hot PE matmul (16 matmuls) off the gpsimd chain, input
    DMAs (rrel/riota/rel table) land early so is_equal finishes by ~12us.
  - PE packing order interleaves one-hot, hrT transposes, tail transposes,
    and score matmuls so the PE never stalls on a single dependency chain.
  - tailsT j-major k-inner; score matmul rhs uses a strided (j, b) AP.
  - GW=2 tail tiles per matmul group: group mm time (~2.5us) matches the
    gather arrival rate (~2.24us/2 tiles), pipelining to the last tile.
  - sigmoids on Scalar; out DMAs alternate Sync/Scalar; act tables loaded
    early via a dummy activation.
"""

import sys

if "/opt/trn_rl_repo" not in sys.path:
    sys.path.insert(0, "/opt/trn_rl_repo")

import numpy as np

import concourse.bass as bass
import concourse.tile as tile
from concourse import bacc, mybir

B = 2048
E = 256
N_ENT = 400000
N_REL = 500
NRELPAD = 512
CORES = 8
P = 128

GH, GT = 4, 2              # core grid: 4 head groups x 2 tail groups
M2 = B // GH               # 512 score rows per core
B2 = B // GT               # 1024 score cols per core
NMh = M2 // P              # 4 head tiles
NTt = B2 // P              # 8 tail tiles
NK = E // P                # 2 contraction tiles
NRC = NRELPAD // P         # 4 rel-id chunks
GW = 2                     # tail tiles per matmul group
NG = NTt // GW             # 4 groups

BF16 = mybir.dt.bfloat16
F32 = mybir.dt.float32
I32 = mybir.dt.int32

C_H, C_T = 0, NMh
NCOL = NMh + NTt

N_WARM = 4


def build_nc():
    nc = bacc.Bacc("TRN2", target_bir_lowering=False, debug=False, num_devices=CORES)

    idx = nc.dram_tensor("idx", [P, NCOL], I32, kind="ExternalInput").ap()
    identity = nc.dram_tensor("identity", [P, P], BF16, kind="ExternalInput").ap()
    table = nc.dram_tensor(
        "table", [N_ENT + NRELPAD, E], BF16, kind="ExternalInput"
    ).ap()
    riota = nc.dram_tensor("riota", [P, 1], F32, kind="ExternalInput").ap()
    rrel = nc.dram_tensor("rrel", [P, NRC * M2], BF16, kind="ExternalInput").ap()
    # score is written in sbuf-dump layout [128, (g i c)] and reassembled
    # host-side: contiguous per-partition runs -> 128 big DMA descriptors
    # per out instead of 512 row-scattered ones.
    score = nc.dram_tensor("score", [P, NMh * B2], BF16, kind="ExternalOutput").ap()

    with tile.TileContext(nc) as tc:
        with (
            tc.tile_pool(name="const", bufs=1) as const_pool,
            tc.tile_pool(name="idxp", bufs=1) as idx_pool,
            tc.tile_pool(name="gather", bufs=1) as gather_pool,
            tc.tile_pool(name="big", bufs=1) as big_pool,
            tc.tile_pool(name="outp", bufs=8) as out_pool,
            tc.tile_pool(name="psrel", bufs=2, space="PSUM") as psum_rel,
            tc.tile_pool(name="pshr", bufs=1, space="PSUM") as psum_hr,
            tc.tile_pool(name="pstl", bufs=1, space="PSUM") as psum_tl,
            tc.tile_pool(name="psmm", bufs=3, space="PSUM") as psum_mm,
        ):
            idx_sb = idx_pool.tile([P, NCOL], I32)
            nc.sync.dma_start(idx_sb[:], idx[:])
            rrel_sb = gather_pool.tile([P, NRC * M2], BF16, tag="rrel")
            nc.sync.dma_start(rrel_sb[:], rrel[:])
            ident = const_pool.tile([P, P], BF16)
            nc.scalar.dma_start(ident[:], identity[:])

            # trigger the sigmoid act-table load early, while Scalar is idle
            dummy = idx_pool.tile([P, 1], BF16, tag="dummy")
            nc.scalar.activation(
                dummy[:], ident[:, 0:1], mybir.ActivationFunctionType.Sigmoid
            )

            # one-hot inputs (all land well before is_equal)
            rel_sb = gather_pool.tile([P, NRC * E], BF16, tag="rel_sb")
            rel_view = table[N_ENT : N_ENT + NRELPAD, :].rearrange(
                "(c p) e -> p c e", p=P
            )
            nc.sync.dma_start(rel_sb[:], rel_view)
            riota_sb = idx_pool.tile([P, 1], F32, tag="riota")
            nc.scalar.dma_start(riota_sb[:], riota[:])

            def g_single(dst, col):
                nc.gpsimd.indirect_dma_start(
                    out=dst,
                    out_offset=None,
                    in_=table[:],
                    in_offset=bass.IndirectOffsetOnAxis(
                        ap=idx_sb[:, col : col + 1], axis=0
                    ),
                )

            # ---- gathers: first tail group, then heads (hr chain), then rest.
            # t0-t3 first so T(g0)/mm(g0) start early and the early gathers'
            # DMA sems are consumed before the ring wraps.
            heads = gather_pool.tile([P, NMh * E], BF16, tag="heads")
            tails = big_pool.tile([P, NTt * E], BF16, tag="tails")
            for j in range(2):
                g_single(tails[:, j * E : (j + 1) * E], C_T + j)
            for i in range(NMh):
                g_single(heads[:, i * E : (i + 1) * E], C_H + i)
            for j in range(2, NTt):
                g_single(tails[:, j * E : (j + 1) * E], C_T + j)

            onehot = gather_pool.tile([P, NRC * M2], BF16, tag="onehot")
            nc.vector.tensor_scalar(
                onehot[:], rrel_sb[:], riota_sb[:], None,
                mybir.AluOpType.is_equal,
            )

            rels = gather_pool.tile([P, NMh * E], BF16, tag="rels")
            hr = gather_pool.tile([P, NMh * E], BF16, tag="hr")
            hrT = gather_pool.tile([P, NMh * NK * P], BF16, tag="hrT")
            rel_ps = {}

            def oh_mm(m):
                rel_ps[m] = psum_rel.tile([P, E], F32, tag="psrel", name=f"relps{m}")
                for t in range(NRC):
                    nc.tensor.matmul(
                        rel_ps[m][:],
                        lhsT=onehot[:, t * M2 + m * P : t * M2 + (m + 1) * P],
                        rhs=rel_sb[:, t * E : (t + 1) * E],
                        start=(t == 0),
                        stop=(t == NRC - 1),
                    )

            def hr_chain(m):
                # DVE: rel copy + hr mult for head tile m
                nc.vector.tensor_copy(
                    rels[:, m * E : (m + 1) * E], rel_ps[m][:]
                )
                nc.vector.tensor_mul(
                    hr[:, m * E : (m + 1) * E],
                    heads[:, m * E : (m + 1) * E],
                    rels[:, m * E : (m + 1) * E],
                )

            def hrT_mm(m):
                pst = psum_hr.tile([P, NK * P], BF16, tag="pshr", name=f"psthr{m}")
                for k in range(NK):
                    nc.tensor.transpose(
                        pst[:, k * P : (k + 1) * P],
                        hr[:, m * E + k * P : m * E + (k + 1) * P],
                        ident[:],
                    )
                nc.vector.tensor_copy(
                    hrT[:, m * NK * P : (m + 1) * NK * P], pst[:]
                )

            # PE packing: one-hot(m) then hrT(m-1) so the DVE hr chain for m-1
            # completes while one-hot(m) streams.
            oh_mm(0)
            hr_chain(0)
            oh_mm(1)
            hr_chain(1)
            hrT_mm(0)
            oh_mm(2)
            hr_chain(2)
            hrT_mm(1)
            oh_mm(3)
            hr_chain(3)
            hrT_mm(2)
            hrT_mm(3)

            # ---- tails: per-group PE transpose + score matmul ----
            tailsT = big_pool.tile([P, NTt * NK * P], BF16, tag="tailsT")
            tt_v = tailsT[:].rearrange("p (j k b) -> p j k b", j=NTt, k=NK)

            WIDTHS = [2, 2, 2, 1, 1]
            STARTS = [0, 2, 4, 6, 7]

            def tails_T(g):
                w, j0 = WIDTHS[g], STARTS[g]
                pst = psum_tl.tile([P, w * NK * P], BF16, tag="pstl", name=f"pst_{g}")
                for jj in range(w):
                    for k in range(NK):
                        c = jj * NK + k
                        j = j0 + jj
                        nc.tensor.transpose(
                            pst[:, c * P : (c + 1) * P],
                            tails[:, j * E + k * P : j * E + (k + 1) * P],
                            ident[:],
                        )
                nc.vector.tensor_copy(
                    tailsT[:, j0 * NK * P : (j0 + w) * NK * P], pst[:]
                )

            def score_mm(g):
                w, j0 = WIDTHS[g], STARTS[g]
                o_tile = out_pool.tile(
                    [P, NMh * w * P], BF16, tag="out", name=f"out_{g}"
                )
                if w == 1:
                    # merged psum: all 4 head tiles in one bank (one zero
                    # region), single activation on the tail-critical chain
                    psmm = psum_mm.tile([P, NMh * P], F32, tag="psmm", name=f"mm_{g}")
                    for i in range(NMh):
                        for k in range(NK):
                            c = i * NK + k
                            nc.tensor.matmul(
                                psmm[:, i * P : (i + 1) * P],
                                lhsT=hrT[:, c * P : (c + 1) * P],
                                rhs=tt_v[:, j0 : j0 + w, k, :],
                                start=(i == 0 and k == 0),
                                stop=(i == NMh - 1 and k == NK - 1),
                                skip_group_check=True,
                            )
                    nc.scalar.activation(
                        o_tile[:], psmm[:], mybir.ActivationFunctionType.Sigmoid
                    )
                else:
                    for i in range(NMh):
                        psmm = psum_mm.tile(
                            [P, w * P], F32, tag="psmm", name=f"mm_{g}_{i}"
                        )
                        for k in range(NK):
                            c = i * NK + k
                            nc.tensor.matmul(
                                psmm[:],
                                lhsT=hrT[:, c * P : (c + 1) * P],
                                rhs=tt_v[:, j0 : j0 + w, k, :],
                                start=(k == 0),
                                stop=(k == NK - 1),
                            )
                        nc.scalar.activation(
                            o_tile[:, i * w * P : (i + 1) * w * P],
                            psmm[:],
                            mybir.ActivationFunctionType.Sigmoid,
                        )
                # one contiguous DMA per group (dump layout)
                off = NMh * j0 * P
                out_eng = nc.scalar if g == len(WIDTHS) - 1 else nc.sync
                out_eng.dma_start(
                    score[:, off : off + NMh * w * P], o_tile[:]
                )

            # pipeline: transpose group g+1 while group g matmuls
            tails_T(0)
            score_mm(0)
            tails_T(1)
            score_mm(1)
            tails_T(2)
            score_mm(2)
            tails_T(3)
            score_mm(3)
            tails_T(4)
            score_mm(4)

    nc.compile()
    return nc


_NC = None


def _get_nc():
    global _NC
    if _NC is None:
        _NC = build_nc()
    return _NC


_TABLE_CACHE = {}


def _make_table(ent_emb, rel_emb):
    import ml_dtypes

    key = (id(ent_emb), id(rel_emb))
    if key in _TABLE_CACHE:
        return _TABLE_CACHE[key]
    ent = np.asarray(ent_emb)
    rel_np = np.asarray(rel_emb)
    rel_diag = rel_np[:, np.arange(E), np.arange(E)]
    tbl = np.zeros((N_ENT + NRELPAD, E), dtype=ml_dtypes.bfloat16)
    tbl[:N_ENT] = ent.astype(ml_dtypes.bfloat16)
    tbl[N_ENT : N_ENT + N_REL] = rel_diag.astype(ml_dtypes.bfloat16)
    _TABLE_CACHE.clear()
    _TABLE_CACHE[key] = tbl
    return tbl


def make_in_maps(batch_h, batch_t, batch_r, ent_emb, rel_emb):
    import ml_dtypes

    h = np.ascontiguousarray(np.asarray(batch_h), dtype=np.int32)
    t = np.ascontiguousarray(np.asarray(batch_t), dtype=np.int32)
    r = np.ascontiguousarray(np.asarray(batch_r), dtype=np.int32)
    tbl = _make_table(ent_emb, rel_emb)
    identity = np.eye(P, dtype=ml_dtypes.bfloat16)
    riota = np.arange(P, dtype=np.float32).reshape(P, 1)

    in_maps = []
    for c in range(CORES):
        hg, tg = c // GT, c % GT
        h_sl = h[hg * M2 : (hg + 1) * M2]
        t_sl = t[tg * B2 : (tg + 1) * B2]
        r_sl = r[hg * M2 : (hg + 1) * M2]
        idx_all = np.concatenate(
            [h_sl.reshape(NMh, P).T, t_sl.reshape(NTt, P).T], axis=1
        )
        rr = np.concatenate([r_sl - P * tt for tt in range(NRC)], axis=0)
        in_maps.append(
            {
                "idx": np.ascontiguousarray(idx_all),
                "identity": identity,
                "table": tbl,
                "riota": riota,
                "rrel": np.ascontiguousarray(
                    np.broadcast_to(rr[None, :], (P, NRC * M2)).astype(
                        ml_dtypes.bfloat16
                    )
                ),
            }
        )
    return in_maps


SCORE_WIDTHS = [2, 2, 2, 1, 1]
SCORE_STARTS = [0, 2, 4, 6, 7]


def unpack_score(dump):
    """[128, (g i c)] dump -> [M2, B2] score block."""
    out = np.empty((M2, B2), dtype=dump.dtype)
    for w, j0 in zip(SCORE_WIDTHS, SCORE_STARTS):
        off = NMh * j0 * P
        for i in range(NMh):
            out[i * P : (i + 1) * P, j0 * P : (j0 + w) * P] = dump[
                :, off + i * w * P : off + (i + 1) * w * P
            ]
    return out


def run(batch_h, batch_t, batch_r, ent_emb, rel_emb, trace=False, tmpdir=None):
    from concourse.bass_utils import run_bass_kernel_spmd

    nc = _get_nc()
    in_maps = make_in_maps(batch_h, batch_t, batch_r, ent_emb, rel_emb)
    kwargs = {}
    if trace:
        kwargs = {"trace": True, "tmpdir": tmpdir}
    res = run_bass_kernel_spmd(nc, in_maps, core_ids=list(range(CORES)), **kwargs)
    score = np.zeros((B, B), dtype=np.float32)
    for c in range(CORES):
        hg, tg = c // GT, c % GT
        blk = unpack_score(np.asarray(res.results[c]["score"]))
        score[hg * M2 : (hg + 1) * M2, tg * B2 : (tg + 1) * B2] = blk.astype(
            np.float32
        )
    return score, res


def kernel(batch_h, batch_t, batch_r, ent_emb, rel_emb):
    score, _ = run(batch_h, batch_t, batch_r, ent_emb, rel_emb)
    return score


# revision 18
# speedup vs baseline: 1.0371x; 1.0371x over previous
"""DistMult scoring kernel v6 for Trainium2 (8 NeuronCores, SPMD 4x2 grid).

score = sigmoid( (ent_emb[h] * diag(rel_emb[r])) @ ent_emb[t].T )

v6 reshards the batch over a 4x2 core grid (4 head-groups x 2 tail-groups):
each core computes a [512, 1024] score block, gathering 4 head tiles + 8
tail tiles = 12 SWDGE calls/core (vs 18 for the 1x8 row sharding). The
SWDGE fixed overhead (~1.1us/call serialized on GpSimd) is THE wall, and
sqrt-sharding minimizes calls: 16/gh + 16/gt is minimal at (4,2).

Schedule (from v4/v5 trace analysis):
  - rels via one-hot PE matmul (16 matmuls) off the gpsimd chain, input
    DMAs (rrel/riota/rel table) land early so is_equal finishes by ~12us.
  - PE packing order interleaves one-hot, hrT transposes, tail transposes,
    and score matmuls so the PE never stalls on a single dependency chain.
  - tailsT j-major k-inner; score matmul rhs uses a strided (j, b) AP.
  - GW=2 tail tiles per matmul group: group mm time (~2.5us) matches the
    gather arrival rate (~2.24us/2 tiles), pipelining to the last tile.
  - sigmoids on Scalar; out DMAs alternate Sync/Scalar; act tables loaded
    early via a dummy activation.
"""

import sys

if "/opt/trn_rl_repo" not in sys.path:
    sys.path.insert(0, "/opt/trn_rl_repo")

import numpy as np

import concourse.bass as bass
import concourse.tile as tile
from concourse import bacc, mybir

B = 2048
E = 256
N_ENT = 400000
N_REL = 500
NRELPAD = 512
CORES = 8
P = 128

GH, GT = 4, 2              # core grid: 4 head groups x 2 tail groups
M2 = B // GH               # 512 score rows per core
B2 = B // GT               # 1024 score cols per core
NMh = M2 // P              # 4 head tiles
NTt = B2 // P              # 8 tail tiles
NK = E // P                # 2 contraction tiles
NRC = NRELPAD // P         # 4 rel-id chunks
GW = 2                     # tail tiles per matmul group
NG = NTt // GW             # 4 groups

BF16 = mybir.dt.bfloat16
F32 = mybir.dt.float32
I32 = mybir.dt.int32

C_H, C_T = 0, NMh
NCOL = NMh + NTt

N_WARM = 4


def build_nc():
    nc = bacc.Bacc("TRN2", target_bir_lowering=False, debug=False, num_devices=CORES)

    idx = nc.dram_tensor("idx", [P, NCOL], I32, kind="ExternalInput").ap()
    identity = nc.dram_tensor("identity", [P, P], BF16, kind="ExternalInput").ap()
    table = nc.dram_tensor(
        "table", [N_ENT + NRELPAD, E], BF16, kind="ExternalInput"
    ).ap()
    riota = nc.dram_tensor("riota", [P, 1], F32, kind="ExternalInput").ap()
    rrel = nc.dram_tensor("rrel", [P, NRC * M2], BF16, kind="ExternalInput").ap()
    # score is written in sbuf-dump layout [128, (g i c)] and reassembled
    # host-side: contiguous per-partition runs -> 128 big DMA descriptors
    # per out instead of 512 row-scattered ones.
    score = nc.dram_tensor("score", [P, NMh * B2], BF16, kind="ExternalOutput").ap()

    with tile.TileContext(nc) as tc:
        with (
            tc.tile_pool(name="const", bufs=1) as const_pool,
            tc.tile_pool(name="idxp", bufs=1) as idx_pool,
            tc.tile_pool(name="gather", bufs=1) as gather_pool,
            tc.tile_pool(name="big", bufs=1) as big_pool,
            tc.tile_pool(name="outp", bufs=8) as out_pool,
            tc.tile_pool(name="psrel", bufs=2, space="PSUM") as psum_rel,
            tc.tile_pool(name="pshr", bufs=1, space="PSUM") as psum_hr,
            tc.tile_pool(name="pstl", bufs=1, space="PSUM") as psum_tl,
            tc.tile_pool(name="psmm", bufs=3, space="PSUM") as psum_mm,
        ):
            idx_sb = idx_pool.tile([P, NCOL], I32)
            nc.sync.dma_start(idx_sb[:], idx[:])
            ident = const_pool.tile([P, P], BF16)
            nc.scalar.dma_start(ident[:], identity[:])
            rrel_sb = gather_pool.tile([P, NRC * M2], BF16, tag="rrel")
            nc.scalar.dma_start(rrel_sb[:], rrel[:])

            # trigger the sigmoid act-table load early, while Scalar is idle
            dummy = idx_pool.tile([P, 1], BF16, tag="dummy")
            nc.scalar.activation(
                dummy[:], ident[:, 0:1], mybir.ActivationFunctionType.Sigmoid
            )

            # one-hot inputs (all land well before is_equal)
            rel_sb = gather_pool.tile([P, NRC * E], BF16, tag="rel_sb")
            rel_view = table[N_ENT : N_ENT + NRELPAD, :].rearrange(
                "(c p) e -> p c e", p=P
            )
            nc.sync.dma_start(rel_sb[:], rel_view)
            riota_sb = idx_pool.tile([P, 1], F32, tag="riota")
            nc.scalar.dma_start(riota_sb[:], riota[:])

            def g_single(dst, col):
                nc.gpsimd.indirect_dma_start(
                    out=dst,
                    out_offset=None,
                    in_=table[:],
                    in_offset=bass.IndirectOffsetOnAxis(
                        ap=idx_sb[:, col : col + 1], axis=0
                    ),
                )

            # ---- gathers: first tail group, then heads (hr chain), then rest.
            # t0-t3 first so T(g0)/mm(g0) start early and the early gathers'
            # DMA sems are consumed before the ring wraps.
            heads = gather_pool.tile([P, NMh * E], BF16, tag="heads")
            tails = big_pool.tile([P, NTt * E], BF16, tag="tails")
            for j in range(2):
                g_single(tails[:, j * E : (j + 1) * E], C_T + j)
            for i in range(NMh):
                g_single(heads[:, i * E : (i + 1) * E], C_H + i)
            for j in range(2, NTt):
                g_single(tails[:, j * E : (j + 1) * E], C_T + j)

            onehot = gather_pool.tile([P, NRC * M2], BF16, tag="onehot")
            nc.vector.tensor_scalar(
                onehot[:], rrel_sb[:], riota_sb[:], None,
                mybir.AluOpType.is_equal,
            )

            rels = gather_pool.tile([P, NMh * E], BF16, tag="rels")
            hr = gather_pool.tile([P, NMh * E], BF16, tag="hr")
            hrT = gather_pool.tile([P, NMh * NK * P], BF16, tag="hrT")
            rel_ps = {}

            def oh_mm(m):
                rel_ps[m] = psum_rel.tile([P, E], F32, tag="psrel", name=f"relps{m}")
                for t in range(NRC):
                    nc.tensor.matmul(
                        rel_ps[m][:],
                        lhsT=onehot[:, t * M2 + m * P : t * M2 + (m + 1) * P],
                        rhs=rel_sb[:, t * E : (t + 1) * E],
                        start=(t == 0),
                        stop=(t == NRC - 1),
                    )

            def hr_chain(m):
                # DVE: rel copy + hr mult for head tile m
                nc.vector.tensor_copy(
                    rels[:, m * E : (m + 1) * E], rel_ps[m][:]
                )
                nc.vector.tensor_mul(
                    hr[:, m * E : (m + 1) * E],
                    heads[:, m * E : (m + 1) * E],
                    rels[:, m * E : (m + 1) * E],
                )

            def hrT_mm(m):
                pst = psum_hr.tile([P, NK * P], BF16, tag="pshr", name=f"psthr{m}")
                for k in range(NK):
                    nc.tensor.transpose(
                        pst[:, k * P : (k + 1) * P],
                        hr[:, m * E + k * P : m * E + (k + 1) * P],
                        ident[:],
                    )
                nc.vector.tensor_copy(
                    hrT[:, m * NK * P : (m + 1) * NK * P], pst[:]
                )

            # PE packing: one-hot(m) then hrT(m-1) so the DVE hr chain for m-1
            # completes while one-hot(m) streams.
            oh_mm(0)
            hr_chain(0)
            oh_mm(1)
            hr_chain(1)
            hrT_mm(0)
            oh_mm(2)
            hr_chain(2)
            hrT_mm(1)
            oh_mm(3)
            hr_chain(3)
            hrT_mm(2)
            hrT_mm(3)

            # ---- tails: per-group PE transpose + score matmul ----
            tailsT = big_pool.tile([P, NTt * NK * P], BF16, tag="tailsT")
            tt_v = tailsT[:].rearrange("p (j k b) -> p j k b", j=NTt, k=NK)

            WIDTHS = [2, 2, 2, 1, 1]
            STARTS = [0, 2, 4, 6, 7]

            def tails_T(g):
                w, j0 = WIDTHS[g], STARTS[g]
                pst = psum_tl.tile([P, w * NK * P], BF16, tag="pstl", name=f"pst_{g}")
                for jj in range(w):
                    for k in range(NK):
                        c = jj * NK + k
                        j = j0 + jj
                        nc.tensor.transpose(
                            pst[:, c * P : (c + 1) * P],
                            tails[:, j * E + k * P : j * E + (k + 1) * P],
                            ident[:],
                        )
                nc.vector.tensor_copy(
                    tailsT[:, j0 * NK * P : (j0 + w) * NK * P], pst[:]
                )

            def score_mm(g):
                w, j0 = WIDTHS[g], STARTS[g]
                o_tile = out_pool.tile(
                    [P, NMh * w * P], BF16, tag="out", name=f"out_{g}"
                )
                if w == 1:
                    # merged psum: all 4 head tiles in one bank (one zero
                    # region), single activation on the tail-critical chain
                    psmm = psum_mm.tile([P, NMh * P], F32, tag="psmm", name=f"mm_{g}")
                    for i in range(NMh):
                        for k in range(NK):
                            c = i * NK + k
                            nc.tensor.matmul(
                                psmm[:, i * P : (i + 1) * P],
                                lhsT=hrT[:, c * P : (c + 1) * P],
                                rhs=tt_v[:, j0 : j0 + w, k, :],
                                start=(i == 0 and k == 0),
                                stop=(i == NMh - 1 and k == NK - 1),
                                skip_group_check=True,
                            )
                    nc.scalar.activation(
                        o_tile[:], psmm[:], mybir.ActivationFunctionType.Sigmoid
                    )
                else:
                    for i in range(NMh):
                        psmm = psum_mm.tile(
                            [P, w * P], F32, tag="psmm", name=f"mm_{g}_{i}"
                        )
                        for k in range(NK):
                            c = i * NK + k
                            nc.tensor.matmul(
                                psmm[:],
                                lhsT=hrT[:, c * P : (c + 1) * P],
                                rhs=tt_v[:, j0 : j0 + w, k, :],
                                start=(k == 0),
                                stop=(k == NK - 1),
                            )
                        nc.scalar.activation(
                            o_tile[:, i * w * P : (i + 1) * w * P],
                            psmm[:],
                            mybir.ActivationFunctionType.Sigmoid,
                        )
                # one contiguous DMA per group (dump layout)
                off = NMh * j0 * P
                out_eng = nc.scalar if g == len(WIDTHS) - 1 else nc.sync
                out_eng.dma_start(
                    score[:, off : off + NMh * w * P], o_tile[:]
                )

            # pipeline: transpose group g+1 while group g matmuls
            tails_T(0)
            score_mm(0)
            tails_T(1)
            score_mm(1)
            tails_T(2)
            score_mm(2)
            tails_T(3)
            score_mm(3)
            tails_T(4)
            score_mm(4)

    nc.compile()
    return nc


_NC = None


def _get_nc():
    global _NC
    if _NC is None:
        _NC = build_nc()
    return _NC


_TABLE_CACHE = {}


def _make_table(ent_emb, rel_emb):
    import ml_dtypes

    key = (id(ent_emb), id(rel_emb))
    if key in _TABLE_CACHE:
        return _TABLE_CACHE[key]
    ent = np.asarray(ent_emb)
    rel_np = np.asarray(rel_emb)
    rel_diag = rel_np[:, np.arange(E), np.arange(E)]
    tbl = np.zeros((N_ENT + NRELPAD, E), dtype=ml_dtypes.bfloat16)
    tbl[:N_ENT] = ent.astype(ml_dtypes.bfloat16)
    tbl[N_ENT : N_ENT + N_REL] = rel_diag.astype(ml_dtypes.bfloat16)
    _TABLE_CACHE.clear()
    _TABLE_CACHE[key] = tbl
    return tbl


def make_in_maps(batch_h, batch_t, batch_r, ent_emb, rel_emb):
    import ml_dtypes

    h = np.ascontiguousarray(np.asarray(batch_h), dtype=np.int32)
    t = np.ascontiguousarray(np.asarray(batch_t), dtype=np.int32)
    r = np.ascontiguousarray(np.asarray(batch_r), dtype=np.int32)
    tbl = _make_table(ent_emb, rel_emb)
    identity = np.eye(P, dtype=ml_dtypes.bfloat16)
    riota = np.arange(P, dtype=np.float32).reshape(P, 1)

    in_maps = []
    for c in range(CORES):
        hg, tg = c // GT, c % GT
        h_sl = h[hg * M2 : (hg + 1) * M2]
        t_sl = t[tg * B2 : (tg + 1) * B2]
        r_sl = r[hg * M2 : (hg + 1) * M2]
        idx_all = np.concatenate(
            [h_sl.reshape(NMh, P).T, t_sl.reshape(NTt, P).T], axis=1
        )
        rr = np.concatenate([r_sl - P * tt for tt in range(NRC)], axis=0)
        in_maps.append(
            {
                "idx": np.ascontiguousarray(idx_all),
                "identity": identity,
                "table": tbl,
                "riota": riota,
                "rrel": np.ascontiguousarray(
                    np.broadcast_to(rr[None, :], (P, NRC * M2)).astype(
                        ml_dtypes.bfloat16
                    )
                ),
            }
        )
    return in_maps


SCORE_WIDTHS = [2, 2, 2, 1, 1]
SCORE_STARTS = [0, 2, 4, 6, 7]


def unpack_score(dump):
    """[128, (g i c)] dump -> [M2, B2] score block."""
    out = np.empty((M2, B2), dtype=dump.dtype)
    for w, j0 in zip(SCORE_WIDTHS, SCORE_STARTS):
        off = NMh * j0 * P
        for i in range(NMh):
            out[i * P : (i + 1) * P, j0 * P : (j0 + w) * P] = dump[
                :, off + i * w * P : off + (i + 1) * w * P
            ]
    return out


def run(batch_h, batch_t, batch_r, ent_emb, rel_emb, trace=False, tmpdir=None):
    from concourse.bass_utils import run_bass_kernel_spmd

    nc = _get_nc()
    in_maps = make_in_maps(batch_h, batch_t, batch_r, ent_emb, rel_emb)
    kwargs = {}
    if trace:
        kwargs = {"trace": True, "tmpdir": tmpdir}
    res = run_bass_kernel_spmd(nc, in_maps, core_ids=list(range(CORES)), **kwargs)
    score = np.zeros((B, B), dtype=np.float32)
    for c in range(CORES):
        hg, tg = c // GT, c % GT
        blk = unpack_score(np.asarray(res.results[c]["score"]))
        score[hg * M2 : (hg + 1) * M2, tg * B2 : (tg + 1) * B2] = blk.astype(
            np.float32
        )
    return score, res


def kernel(batch_h, batch_t, batch_r, ent_emb, rel_emb):
    score, _ = run(batch_h, batch_t, batch_r, ent_emb, rel_emb)
    return score
